# revision 1
# baseline (speedup 1.0000x reference)
"""InterfaceBoundaryLoss Trainium2 kernel.

Data-parallel over batch across 8 NeuronCores.  The [H,W] interface mask is
analyzed on the host and covered with a small set of rectangular "boxes";
the device only streams / computes the boxed regions (the mask is a thin
circle, so this is ~4% of the dense frame).  Per box, all 8 local batches
are fused into the free dimension of [rows, 8*w] tiles.

Math (per batch b, cell (i,j) with mask m=1):
  pot += (phi1-phi2)^2
  der += (EPS1*d1 - EPS2*d2)^2,  dk = nx*dpx_k + ny*dpy_k
Let psi = 0.025*phi2 - phi1 = -(80*phi1 - 2*phi2)/80.  Then
  EPS1*d1 - EPS2*d2 = -40000*(nx*Dx(psi) + ny*Dy(psi))
with Dx/Dy the raw central differences.  So with host fields
  A = 40000*m*nx, B = 40000*m*ny      (zero off-mask)
  der = sum((A*Dx(psi) + B*Dy(psi))^2)
Dy is computed on the TensorEngine via a banded +/-1 stationary matrix,
Dx on the VectorEngine via shifted views.  Square+sum reductions run on
the ScalarEngine (activation Square with accum_out); the pot path runs
on GpSimd.  Host sums per-partition partials in float64.

Mask cells on the frame border (edge-padding semantics) are computed
exactly on the host (none for the reference circle mask).
"""

import sys

for _p in ("/opt/trn_rl_repo",):
    if _p not in sys.path:
        sys.path.append(_p)

import numpy as np
import ml_dtypes

B, H, W = 64, 1024, 1024
EPS1, EPS2 = 80.0, 2.0
DX, DY = 0.001, 0.001
CX, CY = 512.0, 512.0
WEIGHT = 1.0
N_CORES = 8
BPC = B // N_CORES

# "bf16" or "f32" device compute dtype.
DEVICE_DTYPE = "f32"

# set TRACE=True (e.g. from a test harness) to profile the device run;
# LAST_EXEC_NS then holds the measured NEFF execution time.
TRACE = False
LAST_EXEC_NS = None

_FD_CAP = {"f32": 2048, "bf16": 4096}


def _normals(h, w):
    ii = np.arange(h, dtype=np.float64)[:, None]
    jj = np.arange(w, dtype=np.float64)[None, :]
    nx = jj - CX
    ny = ii - CY
    norm = np.sqrt(nx * nx + ny * ny)
    safe = np.where(norm > 0, norm, 1.0)
    return nx / safe, ny / safe


def _cluster(cols, gap):
    """Group sorted col indices into (start, end) inclusive intervals."""
    out = []
    s = p = cols[0]
    for c in cols[1:]:
        if c - p > gap:
            out.append((s, p))
            s = c
        p = c
    out.append((s, p))
    return out


class _Box:
    __slots__ = ("r0", "nrows", "c0", "w", "nb", "ngroups")

    def __init__(self, r0, nrows, c0, w):
        self.r0, self.nrows, self.c0, self.w = int(r0), int(nrows), int(c0), int(w)


def _plan(mask):
    """Cover interior mask cells with boxes.

    Each box loads rows [r0, r0+nrows) x cols [c0, c0+w); cells assigned to
    it are in relative rows [1, nrows-2] and relative cols [1, w-2].
    Returns (boxes, assigned_masks, host_cells) where assigned_masks is the
    per-box bool array [nrows, w] of cells this box owns.
    """
    h, w_ = mask.shape
    border = np.zeros_like(mask)
    border[0, :] = border[-1, :] = True
    border[:, 0] = border[:, -1] = True
    host_cells = mask & border
    core = mask & ~border

    # Recursive cost-driven segmentation: a segment of rows is covered by
    # one box per column-cluster; split the segment in half whenever the
    # two halves' covers are cheaper (box fixed cost ~3000 cyc, ~22 cyc/col).
    def seg_specs(rs, h):
        cols = np.flatnonzero(core[rs : rs + h].any(axis=0))
        if len(cols) == 0:
            return 0.0, []
        clusters = _cluster(cols, gap=17)
        if len(clusters) > 2:
            clusters = [(cols[0], cols[-1])]
        cost = sum(3000.0 + 22.0 * (cb - ca + 10) for ca, cb in clusters)
        return cost, [(rs, h, clusters)]

    def dp(rs, h):
        c0, s0 = seg_specs(rs, h)
        if h <= 2 or not s0:
            return c0, s0
        h1 = h // 2
        ca_, sa = dp(rs, h1)
        cb_, sb = dp(rs + h1, h - h1)
        if ca_ + cb_ < c0:
            return ca_ + cb_, sa + sb
        return c0, s0

    boxes = []
    owned = []
    assigned = np.zeros_like(mask)
    rows_any = np.flatnonzero(core.any(axis=1))
    if len(rows_any):
        r = rows_any[0]
        rmax = rows_any[-1]
        specs = []
        while r <= rmax:
            if not core[r].any():
                r += 1
                continue
            h0 = min(126, rmax + 1 - r)
            _, s = dp(r, h0)
            specs.extend(s)
            r += h0
        for rs, hseg, clusters in specs:
            re_ = rs + hseg
            r = rs
            for ca0, cb0 in clusters:
                # split clusters wider than 498 so box width stays <= 512
                for ca in range(ca0, cb0 + 1, 498):
                    cb = min(ca + 497, cb0)
                    c0 = ca - 2
                    bw = cb + 3 - c0
                    if c0 % 2:
                        c0 -= 1
                        bw += 1
                    bw = -(-bw // 8) * 8
                    if c0 < 0:
                        c0 = 0
                    if c0 + bw > w_:
                        c0 = w_ - bw
                    bx = _Box(r - 1, hseg + 2, c0, bw)
                    sel = np.zeros((bx.nrows, bw), dtype=bool)
                    sub = core[r:re_, ca : cb + 1] & ~assigned[r:re_, ca : cb + 1]
                    sel[1 : 1 + hseg, ca - c0 : cb + 1 - c0] = sub
                    assigned[r:re_, ca : cb + 1] |= sub
                    boxes.append(bx)
                    owned.append(sel)
            r = re_

    leftover = core & ~assigned
    if leftover.any():
        host_cells = host_cells | leftover
        for sel, bx in zip(owned, boxes):
            lv = leftover[bx.r0 : bx.r0 + bx.nrows, bx.c0 : bx.c0 + bx.w]
            sel &= ~lv
    return boxes, owned, host_cells


def _host_contrib(cells_ij, phi1, phi2, nx, ny):
    """Exact (edge-padded) pot/der sums for the given cells, all batches."""
    if len(cells_ij[0]) == 0:
        return 0.0, 0.0
    ii, jj = cells_ij
    p1 = phi1.astype(np.float64)
    p2 = phi2.astype(np.float64)
    d = p1[:, ii, jj] - p2[:, ii, jj]
    pot = float(np.sum(d * d))

    # edge-padded central differences: clamp the *derivative* index
    jc = np.clip(jj, 1, W - 2)
    ic = np.clip(ii, 1, H - 2)

    def dn(p):
        dpx = (p[:, ii, jc + 1] - p[:, ii, jc - 1]) / (2.0 * DX)
        dpy = (p[:, ic + 1, jj] - p[:, ic - 1, jj]) / (2.0 * DY)
        return nx[ii, jj] * dpx + ny[ii, jj] * dpy

    mm = EPS1 * dn(p1) - EPS2 * dn(p2)
    der = float(np.sum(mm * mm))
    return pot, der


def _build_nc(boxes, dt_str, fd_cap):
    from contextlib import ExitStack
    from concourse import bass, bacc, tile, mybir

    mdt = mybir.dt.bfloat16 if dt_str == "bf16" else mybir.dt.float32
    f32 = mybir.dt.float32
    mult = mybir.AluOpType.mult
    sub = mybir.AluOpType.subtract
    SQ = mybir.ActivationFunctionType.Square

    njobs = sum(bx.ngroups for bx in boxes)
    nc = bacc.Bacc(
        "TRN2", target_bir_lowering=False, debug=False, num_devices=N_CORES
    )

    phi1_d = nc.dram_tensor("phi1", [BPC * H, W], mdt, kind="ExternalInput")
    phi2_d = nc.dram_tensor("phi2", [BPC * H, W], mdt, kind="ExternalInput")
    dmat_d = nc.dram_tensor("dmat", [128, 128], mdt, kind="ExternalInput")
    a_ds, b_ds, m_ds = [], [], []
    for k, bx in enumerate(boxes):
        a_ds.append(nc.dram_tensor(f"a{k}", [bx.nrows, bx.w], mdt, kind="ExternalInput"))
        b_ds.append(nc.dram_tensor(f"b{k}", [bx.nrows, bx.w], mdt, kind="ExternalInput"))
        m_ds.append(nc.dram_tensor(f"m{k}", [bx.nrows, bx.w], mdt, kind="ExternalInput"))
    acc_d = nc.dram_tensor("acc", [128, 2 * njobs], f32, kind="ExternalOutput")

    with tile.TileContext(nc) as tc, ExitStack() as ctx:
        io = ctx.enter_context(tc.tile_pool(name="io", bufs=3))
        cst = ctx.enter_context(tc.tile_pool(name="cst", bufs=2))
        mid = ctx.enter_context(tc.tile_pool(name="mid", bufs=2))
        pot_p = ctx.enter_context(tc.tile_pool(name="potp", bufs=2))
        psum = ctx.enter_context(tc.tile_pool(name="psum", bufs=4, space="PSUM"))
        onep = ctx.enter_context(tc.tile_pool(name="onep", bufs=1))

        dm = onep.tile([128, 128], mdt)
        nc.sync.dma_start(dm[:], dmat_d.ap())
        acc = onep.tile([128, 2 * njobs], f32)
        nc.vector.memset(acc[:], 0.0)

        job = 0
        for k, bx in enumerate(boxes):
            nr, w, nb = bx.nrows, bx.w, bx.nb
            fd = nb * w
            at = cst.tile([nr, w], mdt, tag="at")
            nc.sync.dma_start(at[:], a_ds[k].ap())
            bt = cst.tile([nr, w], mdt, tag="bt")
            nc.sync.dma_start(bt[:], b_ds[k].ap())
            mt = cst.tile([nr, w], mdt, tag="mt")
            nc.sync.dma_start(mt[:], m_ds[k].ap())
            a3 = at[:].unsqueeze(1).broadcast_to([nr, nb, w])
            b3 = bt[:].unsqueeze(1).broadcast_to([nr, nb, w])
            m3 = mt[:].unsqueeze(1).broadcast_to([nr, nb, w])

            for g in range(bx.ngroups):
                b0 = g * nb
                f1 = io.tile([nr, fd], mdt, tag="f1")
                f2 = io.tile([nr, fd], mdt, tag="f2")
                for ft, src_d in ((f1, phi1_d), (f2, phi2_d)):
                    src = bass.AP(
                        src_d,
                        (b0 * H + bx.r0) * W + bx.c0,
                        [[W, nr], [H * W, nb], [1, w]],
                    )
                    dst = ft[:].rearrange("p (b w) -> p b w", b=nb)
                    nc.sync.dma_start(dst, src)

                # psi = 0.025*phi2 - phi1
                psi = mid.tile([nr, fd], mdt, tag="psi")
                nc.vector.scalar_tensor_tensor(
                    psi[:], f2[:], 0.025, f1[:], op0=mult, op1=sub
                )

                # dxs[f] = psi[f+2] - psi[f]  (cell at f+1)
                dxs = mid.tile([nr, fd], mdt, tag="dxs")
                nc.vector.tensor_sub(
                    dxs[:, 0 : fd - 2], psi[:, 2:fd], psi[:, 0 : fd - 2]
                )
                nc.vector.memset(dxs[:, fd - 2 : fd], 0.0)
                u = mid.tile([nr, fd], mdt, tag="u")
                nc.vector.tensor_mul(
                    u[:].rearrange("p (b w) -> p b w", b=nb),
                    dxs[:].rearrange("p (b w) -> p b w", b=nb),
                    a3,
                )

                # dy via PE: dy[mi, f] = psi[mi+1, f] - psi[mi-1, f];
                # batch-aligned chunks of gchunk blocks (gchunk*w <= 512)
                v = mid.tile([nr, fd], mdt, tag="v")
                v3 = v[:].rearrange("p (b w) -> p b w", b=nb)
                nc.vector.memset(v3[:, :, w - 1 : w], 0.0)
                gchunk = max(1, 512 // w)
                for j0 in range(0, nb, gchunk):
                    gg = min(gchunk, nb - j0)
                    dy = psum.tile([128, 512], f32, tag="dy")
                    nc.tensor.matmul(
                        dy[:, 0 : gg * w],
                        dm[0:nr, :],
                        psi[:, j0 * w : (j0 + gg) * w],
                        start=True,
                        stop=True,
                    )
                    dy3 = dy[0:nr, 0 : gg * w].rearrange("p (b w) -> p b w", b=gg)
                    nc.vector.tensor_mul(
                        v3[:, j0 : j0 + gg, 0 : w - 1],
                        bt[:].unsqueeze(1).broadcast_to([nr, gg, w])[:, :, 0 : w - 1],
                        dy3[:, :, 1:w],
                    )

                wt = mid.tile([nr, fd], mdt, tag="wt")
                nc.vector.tensor_add(wt[:], u[:], v[:])
                nc.scalar.activation(
                    dxs[:],
                    wt[:],
                    SQ,
                    accum_out=acc[0:nr, njobs + job : njobs + job + 1],
                )

                # pot path on GpSimd
                df = pot_p.tile([nr, fd], mdt, tag="df")
                nc.gpsimd.tensor_sub(df[:], f2[:], f1[:])
                w1 = pot_p.tile([nr, fd], mdt, tag="w1")
                nc.gpsimd.tensor_mul(
                    w1[:].rearrange("p (b w) -> p b w", b=nb),
                    df[:].rearrange("p (b w) -> p b w", b=nb),
                    m3,
                )
                nc.scalar.activation(
                    df[:],
                    w1[:],
                    SQ,
                    accum_out=acc[0:nr, job : job + 1],
                )
                job += 1

        nc.sync.dma_start(acc_d.ap(), acc[:])

    nc.compile()
    return nc


def _prepare(mask):
    """Plan boxes and build all mask-derived constant arrays."""
    nx, ny = _normals(H, W)
    boxes, owned, host_cells = _plan(mask)

    fd_cap = _FD_CAP[DEVICE_DTYPE]
    np_dt = ml_dtypes.bfloat16 if DEVICE_DTYPE == "bf16" else np.float32

    for bx in boxes:
        nb = max(1, min(BPC, fd_cap // bx.w))
        while BPC % nb:
            nb -= 1
        bx.nb = nb
        bx.ngroups = BPC // nb

    consts = {}
    af = 40000.0 * nx
    bf = 40000.0 * ny
    for k, (bx, sel) in enumerate(zip(boxes, owned)):
        rs, cs = slice(bx.r0, bx.r0 + bx.nrows), slice(bx.c0, bx.c0 + bx.w)
        a_box = np.where(sel, af[rs, cs], 0.0)
        b_box = np.where(sel, bf[rs, cs], 0.0)
        # shift left by one col: field[k] = value at col k+1
        a_sh = np.zeros_like(a_box)
        a_sh[:, :-1] = a_box[:, 1:]
        b_sh = np.zeros_like(b_box)
        b_sh[:, :-1] = b_box[:, 1:]
        consts[f"a{k}"] = a_sh.astype(np_dt)
        consts[f"b{k}"] = b_sh.astype(np_dt)
        consts[f"m{k}"] = sel.astype(np_dt)

    dmat = np.zeros((128, 128), dtype=np.float64)
    for mi in range(1, 127):
        dmat[mi + 1, mi] = 1.0
        dmat[mi - 1, mi] = -1.0
    consts["dmat"] = dmat.astype(np_dt)
    return boxes, consts, host_cells, np_dt


_CACHE = {}


def kernel(output_in, output_out, interface_mask):
    from concourse.bass_utils import run_bass_kernel_spmd

    phi1 = np.asarray(output_in).reshape(B, H, W)
    phi2 = np.asarray(output_out).reshape(B, H, W)
    mask = np.asarray(interface_mask).astype(bool)

    n_mask = float(mask.sum())
    if n_mask == 0.0:
        return np.float32(np.nan)

    key = (mask.tobytes(), DEVICE_DTYPE)
    if key not in _CACHE:
        boxes, consts, host_cells, np_dt = _prepare(mask)
        nc = _build_nc(boxes, DEVICE_DTYPE, _FD_CAP[DEVICE_DTYPE]) if boxes else None
        _CACHE[key] = (boxes, consts, host_cells, np_dt, nc)
    boxes, consts, host_cells, np_dt, nc = _CACHE[key]

    pot = der = 0.0
    if nc is not None:
        in_maps = []
        for c in range(N_CORES):
            sl = slice(c * BPC, (c + 1) * BPC)
            m = dict(consts)
            m["phi1"] = np.ascontiguousarray(phi1[sl]).reshape(BPC * H, W).astype(np_dt)
            m["phi2"] = np.ascontiguousarray(phi2[sl]).reshape(BPC * H, W).astype(np_dt)
            in_maps.append(m)
        res = run_bass_kernel_spmd(
            nc, in_maps, core_ids=list(range(N_CORES)), trace=TRACE
        )
        global LAST_EXEC_NS
        LAST_EXEC_NS = res.exec_time_ns
        njobs = sum(bx.ngroups for bx in boxes)
        for r in res.results:
            a = r["acc"].astype(np.float64)
            pot += float(a[:, :njobs].sum())
            der += float(a[:, njobs:].sum())

    if host_cells.any():
        nx, ny = _normals(H, W)
        hp, hd = _host_contrib(np.nonzero(host_cells), phi1, phi2, nx, ny)
        pot += hp
        der += hd

    denom = B * n_mask
    return np.float32(WEIGHT * (pot + der) / denom)



# revision 6
# speedup vs baseline: 2.4635x; 2.4635x over previous
"""InterfaceBoundaryLoss Trainium2 kernel (v2).

Data-parallel over batch across 8 NeuronCores.  The [H,W] interface mask is
covered on the host with boxes hugging the circle: 128-row-tall boxes where
the arc is steep (narrow column clusters) and 32-row bands elsewhere, with
wide clusters split into <=64-col pieces.  32-row boxes are stacked 4-high
in the partition dimension so elementwise work scales with the packed free
width, not the box count.  phi1/phi2 are interleaved host-side into one
[BPC, 2, H, W] array so each box needs a single 3D-AP DMA.

Math per mask cell (i,j):
  pot += (phi1-phi2)^2
  der += (EPS1*d1 - EPS2*d2)^2 = (A*Dx(psi) + B*Dy(psi))^2
with psi = 0.025*phi2 - phi1, A = 40000*m*nx, B = 40000*m*ny (shifted one
col left so layout col k holds cell k+1), Dx/Dy raw central differences.
Dy runs on the TensorEngine via a banded +/-1 matrix (block halo rows are
masked), Dx on the VectorEngine via shifted views.  pot runs on GpSimd.
One Square activation per group accumulates pot+der together; the host
sums partials in float64 and applies WEIGHT/denom.
"""

import sys

for _p in ("/opt/trn_rl_repo",):
    if _p not in sys.path:
        sys.path.append(_p)

import numpy as np
import ml_dtypes

B, H, W = 64, 1024, 1024
EPS1, EPS2 = 80.0, 2.0
DX, DY = 0.001, 0.001
CX, CY = 512.0, 512.0
WEIGHT = 1.0
N_CORES = 8
BPC = B // N_CORES

TALL = 128
SHORT = 32
TALL_MAX_W = 64
WQ = (16, 32, 64)
GAP = 6
SUB_W = 64
NGROUPS = 3

TRACE = False
LAST_EXEC_NS = None


class _Box:
    __slots__ = ("r0", "nr", "c0", "w", "sel", "part0", "f0")

    def __init__(self, r0, nr, c0, w):
        self.r0, self.nr, self.c0, self.w = int(r0), int(nr), int(c0), int(w)
        self.sel = None


def _clusters(cols, gap=GAP):
    out = []
    s = p = cols[0]
    for c in cols[1:]:
        if c - p > gap:
            out.append((s, p))
            s = c
        p = c
    out.append((s, p))
    return out


def _plan(mask):
    h, w_ = mask.shape
    border = np.zeros_like(mask)
    border[0, :] = border[-1, :] = True
    border[:, 0] = border[:, -1] = True
    host_cells = mask & border
    core = mask & ~border
    assigned = np.zeros_like(mask)

    rows_any = np.flatnonzero(core.any(axis=1))
    boxes = []
    if len(rows_any) == 0:
        return boxes, host_cells
    r = int(rows_any[0])
    rmax = int(rows_any[-1])
    while r <= rmax:
        if not core[r].any():
            r += 1
            continue
        emitted = False
        for nr in (TALL, SHORT):
            own_lo = r
            own_hi = min(r + nr - 2, rmax + 1)
            if own_lo - 1 + nr > h:  # box must fit in the frame
                continue
            sub = core[own_lo:own_hi]
            cols = np.flatnonzero(sub.any(axis=0))
            if len(cols) == 0:
                break
            cls = _clusters(cols)
            if nr == TALL and (
                max(cb - ca + 1 for ca, cb in cls) > TALL_MAX_W
                or own_hi - own_lo < 96
            ):
                continue
            for ca, cb in cls:
                span = cb - ca + 1
                net = SUB_W - 4
                npieces = max(1, -(-span // net)) if span > SUB_W - 2 else 1
                for pi in range(npieces):
                    pa = ca + pi * net
                    pb = min(pa + net - 1, cb)
                    if pa > cb:
                        break
                    ww = pb - pa + 3
                    wq = next((q for q in WQ if q >= ww), None)
                    if wq is None:
                        wq = -(-ww // 64) * 64
                    c0 = pa - 1 - (wq - ww) // 2
                    c0 = max(0, min(c0, w_ - wq))
                    bx = _Box(r - 1, nr, c0, wq)
                    sel = np.zeros((nr, wq), dtype=bool)
                    s = (
                        core[own_lo:own_hi, pa : pb + 1]
                        & ~assigned[own_lo:own_hi, pa : pb + 1]
                    )
                    sel[
                        own_lo - bx.r0 : own_hi - bx.r0, pa - c0 : pb + 1 - c0
                    ] = s
                    assigned[own_lo:own_hi, pa : pb + 1] |= s
                    rr_, cc_ = np.nonzero(sel)
                    if len(rr_) == 0:
                        continue
                    assert rr_.min() >= 1 and rr_.max() <= nr - 2
                    assert cc_.min() >= 1 and cc_.max() <= wq - 2
                    bx.sel = sel
                    boxes.append(bx)
            r = own_hi
            emitted = True
            break
        if not emitted:
            r += 1

    leftover = core & ~assigned
    if leftover.any():
        host_cells = host_cells | leftover
    return boxes, host_cells


def _stack(boxes):
    """Tall boxes stand alone; 32-row boxes pack 4-high by equal width.
    Returns (stacks, w_tot); assigns part0/f0 on each box."""
    stacks = []
    shorts = {}
    for bx in boxes:
        if bx.nr == TALL:
            bx.part0 = 0
            stacks.append([bx])
        else:
            shorts.setdefault(bx.w, []).append(bx)
    for wq in sorted(shorts):
        lst = shorts[wq]
        for i in range(0, len(lst), 4):
            grp = lst[i : i + 4]
            for j, bx in enumerate(grp):
                bx.part0 = j * SHORT
            stacks.append(grp)
    f = 0
    for st in stacks:
        for bx in st:
            bx.f0 = f
        f += st[0].w
    return stacks, f


def _normals(h, w):
    ii = np.arange(h, dtype=np.float64)[:, None]
    jj = np.arange(w, dtype=np.float64)[None, :]
    nx = jj - CX
    ny = ii - CY
    norm = np.sqrt(nx * nx + ny * ny)
    safe = np.where(norm > 0, norm, 1.0)
    return nx / safe, ny / safe


def _host_contrib(cells_ij, phi1, phi2, nx, ny):
    if len(cells_ij[0]) == 0:
        return 0.0
    ii, jj = cells_ij
    p1 = phi1.astype(np.float64)
    p2 = phi2.astype(np.float64)
    d = p1[:, ii, jj] - p2[:, ii, jj]
    tot = float(np.sum(d * d))
    jc = np.clip(jj, 1, W - 2)
    ic = np.clip(ii, 1, H - 2)

    def dn(p):
        dpx = (p[:, ii, jc + 1] - p[:, ii, jc - 1]) / (2.0 * DX)
        dpy = (p[:, ic + 1, jj] - p[:, ic - 1, jj]) / (2.0 * DY)
        return nx[ii, jj] * dpx + ny[ii, jj] * dpy

    mm = EPS1 * dn(p1) - EPS2 * dn(p2)
    tot += float(np.sum(mm * mm))
    return tot


def _prepare(mask):
    np_dt = ml_dtypes.bfloat16
    nx, ny = _normals(H, W)
    boxes, host_cells = _plan(mask)
    stacks, w_tot = _stack(boxes)

    af = 40000.0 * nx
    bf = 40000.0 * ny
    # const block [128, 3*w_tot]: A | B | M, stacked per box
    cst = np.zeros((128, 3 * w_tot), dtype=np.float64)
    for bx in boxes:
        rs = slice(bx.r0, bx.r0 + bx.nr)
        cs = slice(bx.c0, bx.c0 + bx.w)
        a = np.where(bx.sel, af[rs, cs], 0.0)
        b = np.where(bx.sel, bf[rs, cs], 0.0)
        a_sh = np.zeros_like(a)
        a_sh[:, :-1] = a[:, 1:]
        b_sh = np.zeros_like(b)
        b_sh[:, :-1] = b[:, 1:]
        ps = slice(bx.part0, bx.part0 + bx.nr)
        cst[ps, bx.f0 : bx.f0 + bx.w] = a_sh
        cst[ps, w_tot + bx.f0 : w_tot + bx.f0 + bx.w] = b_sh
        cst[ps, 2 * w_tot + bx.f0 : 2 * w_tot + bx.f0 + bx.w] = bx.sel

    dmat = np.zeros((128, 128), dtype=np.float64)
    for mi in range(1, 127):
        dmat[mi + 1, mi] = 1.0
        dmat[mi - 1, mi] = -1.0

    consts = {
        "cst": cst.astype(np_dt),
        "dmat": dmat.astype(np_dt),
    }
    # groups: consecutive stacks binned by ~equal free width
    groups = []
    tgt = max(1, -(-w_tot // NGROUPS))
    cur, acc = [], 0
    for st in stacks:
        cur.append(st)
        acc += st[0].w
        if acc >= tgt and len(groups) < NGROUPS - 1:
            groups.append(cur)
            cur, acc = [], 0
    if cur:
        groups.append(cur)
    return boxes, stacks, groups, w_tot, consts, host_cells, np_dt


def _build_nc(stacks, groups, w_tot):
    from contextlib import ExitStack
    from concourse import bass, bacc, tile, mybir

    mdt = mybir.dt.bfloat16
    f32 = mybir.dt.float32
    mult = mybir.AluOpType.mult
    sub = mybir.AluOpType.subtract
    SQ = mybir.ActivationFunctionType.Square

    F8 = 8 * w_tot
    ng = len(groups)

    nc = bacc.Bacc(
        "TRN2", target_bir_lowering=False, debug=False, num_devices=N_CORES
    )
    x_d = nc.dram_tensor("x", [2 * BPC * H, W], mdt, kind="ExternalInput")
    cst_d = nc.dram_tensor("cst", [128, 3 * w_tot], mdt, kind="ExternalInput")
    dmat_d = nc.dram_tensor("dmat", [128, 128], mdt, kind="ExternalInput")
    acc_d = nc.dram_tensor("acc", [128, ng], f32, kind="ExternalOutput")

    with tile.TileContext(nc) as tc, ExitStack() as ctx:
        onep = ctx.enter_context(tc.tile_pool(name="onep", bufs=1))
        vpool = ctx.enter_context(tc.tile_pool(name="vpool", bufs=3))
        dpool = ctx.enter_context(tc.tile_pool(name="dpool", bufs=3))
        psum = ctx.enter_context(tc.tile_pool(name="psum", bufs=4, space="PSUM"))

        X = onep.tile([128, 16 * w_tot], mdt)
        psi = onep.tile([128, F8], mdt)
        dxs = onep.tile([128, F8], mdt)
        sq = onep.tile([128, 2 * F8], mdt)
        cstt = onep.tile([128, 3 * w_tot], mdt)
        dm = onep.tile([128, 128], mdt)
        acc = onep.tile([128, ng], f32)

        nc.sync.dma_start(cstt[:], cst_d.ap())
        nc.sync.dma_start(dm[:], dmat_d.ap())
        nc.vector.memset(acc[:], 0.0)

        # memset empty stack slots of X so psi/df stay finite
        for st in stacks:
            used = sum(bx.nr for bx in st)
            if used < 128:
                nc.vector.memset(
                    X[used:128, 16 * st[0].f0 : 16 * (st[0].f0 + st[0].w)], 0.0
                )

        # input DMAs, alternating HWDGE queues, in group order
        qi = 0
        for gstacks in groups:
            for st in gstacks:
                for bx in st:
                    src = bass.AP(
                        x_d,
                        bx.r0 * W + bx.c0,
                        [[W, bx.nr], [H * W, 2 * BPC], [1, bx.w]],
                    )
                    dst = (
                        X[bx.part0 : bx.part0 + bx.nr, 16 * bx.f0 : 16 * (bx.f0 + bx.w)]
                        .rearrange("p (bt w) -> p bt w", bt=2 * BPC)
                    )
                    eng = nc.sync if qi % 2 == 0 else nc.scalar
                    eng.dma_start(dst, src)
                    qi += 1

        off = 0  # running col offset into sq (per group: [wt_g | dfm_g])
        for g, gstacks in enumerate(groups):
            fg0 = gstacks[0][0].f0
            wg = sum(st[0].w for st in gstacks)
            a0, b0, m0 = fg0, w_tot + fg0, 2 * w_tot + fg0

            # psi/df per stack: psi = 0.025*f2 - f1 (DVE), df = f1 - f2 (Pool)
            for st in gstacks:
                f0, w = st[0].f0, st[0].w
                xv = X[:, 16 * f0 : 16 * (f0 + w)].rearrange(
                    "p (b tw) -> p b tw", b=BPC
                )
                xt0 = xv[:, :, 0:w]
                xt1 = xv[:, :, w : 2 * w]
                pview = psi[:, 8 * f0 : 8 * (f0 + w)].rearrange(
                    "p (b w) -> p b w", b=BPC
                )
                nc.vector.scalar_tensor_tensor(
                    pview, xt1, 0.025, xt0, op0=mult, op1=sub
                )
                dft = dpool.tile([128, 8 * w], mdt, tag="df")
                nc.gpsimd.tensor_sub(
                    dft[:].rearrange("p (b w) -> p b w", b=BPC), xt0, xt1
                )
                mt = cstt[:, m0 : m0 + w]
                st_dfm0 = off + wg * 8 + 8 * (f0 - fg0)
                nc.gpsimd.tensor_mul(
                    sq[:, st_dfm0 : st_dfm0 + 8 * w].rearrange(
                        "p (b w) -> p b w", b=BPC
                    ),
                    dft[:].rearrange("p (b w) -> p b w", b=BPC),
                    mt.unsqueeze(1).broadcast_to([128, BPC, w]),
                )
                m0 += w

            # dxs over the group's psi range (tail 2 cols memset; masked by A=0)
            ga, gb = 8 * fg0, 8 * (fg0 + wg)
            nc.vector.tensor_sub(
                dxs[:, ga : gb - 2], psi[:, ga + 2 : gb], psi[:, ga : gb - 2]
            )
            nc.vector.memset(dxs[:, gb - 2 : gb], 0.0)

            # per stack: dy matmul, u, v, wt
            for st in gstacks:
                f0, w = st[0].f0, st[0].w
                fa = 8 * f0
                at = cstt[:, a0 : a0 + w]
                bt = cstt[:, b0 : b0 + w]
                a0 += w
                b0 += w
                dy = psum.tile([128, 8 * w], f32, tag="dy")
                nc.tensor.matmul(
                    dy[:], dm[:], psi[:, fa : fa + 8 * w], start=True, stop=True
                )
                st_wt0 = off + 8 * (f0 - fg0)
                wview = sq[:, st_wt0 : st_wt0 + 8 * w].rearrange(
                    "p (b w) -> p b w", b=BPC
                )
                # u = A * dxs  (layout col k = cell k+1)
                nc.vector.tensor_mul(
                    wview,
                    dxs[:, fa : fa + 8 * w].rearrange("p (b w) -> p b w", b=BPC),
                    at.unsqueeze(1).broadcast_to([128, BPC, w]),
                )
                # v = B * dy (dy shifted 1 col to align at cell k+1)
                vt = vpool.tile([128, 8 * w], mdt, tag="v")
                v3 = vt[:].rearrange("p (b w) -> p b w", b=BPC)
                nc.vector.memset(v3[:, :, w - 1 : w], 0.0)
                nc.vector.tensor_mul(
                    v3[:, :, 0 : w - 1],
                    bt.unsqueeze(1).broadcast_to([128, BPC, w])[:, :, 0 : w - 1],
                    dy[:].rearrange("p (b w) -> p b w", b=BPC)[:, :, 1:w],
                )
                # wt = u + v
                nc.vector.tensor_add(
                    sq[:, st_wt0 : st_wt0 + 8 * w],
                    sq[:, st_wt0 : st_wt0 + 8 * w],
                    vt[:],
                )

            # one Square+accum over [wt_g | dfm_g]; X's group region is dead
            # by now (psi/df consumed it) so it serves as the trash output.
            nc.scalar.activation(
                X[:, 16 * fg0 : 16 * (fg0 + wg)],
                sq[:, off : off + 16 * wg],
                SQ,
                accum_out=acc[:, g : g + 1],
            )
            off += 16 * wg

        nc.sync.dma_start(acc_d.ap(), acc[:])

    nc.compile()
    return nc


_CACHE = {}


def kernel(output_in, output_out, interface_mask):
    from concourse.bass_utils import run_bass_kernel_spmd

    phi1 = np.asarray(output_in).reshape(B, H, W)
    phi2 = np.asarray(output_out).reshape(B, H, W)
    mask = np.asarray(interface_mask).astype(bool)

    n_mask = float(mask.sum())
    if n_mask == 0.0:
        return np.float32(np.nan)

    key = mask.tobytes()
    if key not in _CACHE:
        boxes, stacks, groups, w_tot, consts, host_cells, np_dt = _prepare(mask)
        nc = _build_nc(stacks, groups, w_tot) if boxes else None
        _CACHE[key] = (stacks, groups, w_tot, consts, host_cells, np_dt, nc)
    stacks, groups, w_tot, consts, host_cells, np_dt, nc = _CACHE[key]

    tot = 0.0
    if nc is not None:
        xi = np.empty((B, 2, H, W), dtype=np_dt)
        xi[:, 0] = phi1.astype(np_dt)
        xi[:, 1] = phi2.astype(np_dt)
        in_maps = []
        for c in range(N_CORES):
            m = dict(consts)
            m["x"] = xi[c * BPC : (c + 1) * BPC].reshape(2 * BPC * H, W)
            in_maps.append(m)
        res = run_bass_kernel_spmd(
            nc, in_maps, core_ids=list(range(N_CORES)), trace=TRACE
        )
        global LAST_EXEC_NS
        LAST_EXEC_NS = res.exec_time_ns
        for r in res.results:
            tot += float(r["acc"].astype(np.float64).sum())

    if host_cells.any():
        nx, ny = _normals(H, W)
        tot += _host_contrib(np.nonzero(host_cells), phi1, phi2, nx, ny)

    denom = B * n_mask
    return np.float32(WEIGHT * tot / denom)
